# revision 40
# baseline (speedup 1.0000x reference)
"""Trainium2 Bass kernel for causal multi-head attention (dense transformer).

Problem shapes (hardcoded): x [2,2048,1024], 16 heads x 64 head-dim.
Sharding: data-parallel over batch (2) x tensor-parallel over heads (4/core)
on 8 NeuronCores. Each core computes the partial output (sum over its 4
heads) for one batch element; the host sums the 4 partials per batch and
adds b_O.

All-bf16 datapath (host pre-casts; 3.6e-3 rel err vs the 2e-2 gate),
~173us HW exec vs the 228-262us f32r baseline.  Key mechanisms:
  - bf16 HW-DGE DMAs, no on-device casts; x split over the 3 DMA-capable
    engine rings (~110-150GB/s each, ~310GB/s aggregate = per-core cap)
  - heads packed in pairs on partitions (even head d0-63, odd d64-127);
    scores run as K=64 matmul PAIRS on row groups (0,0)/(64,0) via
    base-partition auto tile_position -> both heads' scores in the time
    of one zero-padded matmul (verified 3ns-apart concurrent on HW).
    AV (K=128) matmuls keep the HAM clock warm despite K=64 scores.
  - causal mask accumulated into score PSUM by an IDEN x TRIM matmul
    (exp(-30000+s)=0).  Moving the mask to a post-exp 0/1 multiply on
    DVE/Pool was 1.5x WORSE (Pool TT is slow + strip critical path).
  - exp on ACT is the conveyor: 80 exps x ~1.24us ([128, 2-head, w]
    strip groups, 2-deep score-PSUM ring, so ACT runs at most 2 exps
    ahead).  The whole schedule is a fine-grained weave: one score
    group, then ~1us of other PE work (sweep chunk / V tile / AV chunk
    / outproj tile), so neither PE nor ACT ever blocks long.
  - phase order: QK sweeps qc0..3 (chunk-major over x DMA) with qc0/qc1
    score groups woven in; V proj (kt quanta) with qc0/1 AV+norm and
    qc2 scores; then qc3 scores with qc2/3 AV, outproj, and output DMA
    chasing per 512-row chunk.
  - V carries a trailing ones column so the softmax denominator falls
    out of the AV matmul; norm: DVE copy (base-64 psum row) -> fast
    reciprocal -> gpsimd partition_broadcast -> DVE multiply
  - PSUM is fully subscribed in every phase: (qk 4 + scores 4), then
    (v 2 + scores 4 + av 2), then (scores 4 + av 2 + outproj 2).
"""

import sys

if "/opt/trn_rl_repo" not in sys.path:
    sys.path.insert(0, "/opt/trn_rl_repo")

import numpy as np
import ml_dtypes

BF = ml_dtypes.bfloat16
B, S, D = 2, 2048, 1024
H, DH = 16, 64
NCORES = 8
NH = 4            # heads per core (2 pairs)
KCH = D // 128    # contraction chunks over model dim
NT = S // 128     # 128-row k tiles
QC = S // 512     # 512-wide q chunks
P = 128
MASK_VAL = -30000.0

_CACHE = {}


def _build_nc():
    import concourse.tile as tile
    from concourse import bacc, mybir

    f32 = mybir.dt.float32
    bf = mybir.dt.bfloat16
    Exp = mybir.ActivationFunctionType.Exp
    Copy = mybir.ActivationFunctionType.Copy
    mult = mybir.AluOpType.mult

    nc = bacc.Bacc("TRN2", target_bir_lowering=False, debug=False,
                   num_devices=NCORES)

    xt_d = nc.dram_tensor("xt", [D, S], bf, kind="ExternalInput").ap()
    wq_d = nc.dram_tensor("wq", [P, KCH * NH * DH], bf, kind="ExternalInput").ap()
    wk_d = nc.dram_tensor("wk", [P, KCH * NH * DH], bf, kind="ExternalInput").ap()
    wv_d = nc.dram_tensor("wv", [P, KCH * NH * DH], bf, kind="ExternalInput").ap()
    wo_d = nc.dram_tensor("wo", [P, 2 * D], bf, kind="ExternalInput").ap()
    bq_d = nc.dram_tensor("bq", [P, 2], f32, kind="ExternalInput").ap()
    bk_d = nc.dram_tensor("bk", [P, 2], f32, kind="ExternalInput").ap()
    bv_d = nc.dram_tensor("bv", [1, NH * DH], bf, kind="ExternalInput").ap()
    ones_d = nc.dram_tensor("ones", [1, P], bf, kind="ExternalInput").ap()
    vones_d = nc.dram_tensor("vones", [P, NT * NH], bf, kind="ExternalInput").ap()
    trim_d = nc.dram_tensor("trim", [P, P], bf, kind="ExternalInput").ap()
    iden_d = nc.dram_tensor("iden", [P, P], bf, kind="ExternalInput").ap()
    out_d = nc.dram_tensor("out", [S, D], bf, kind="ExternalOutput").ap()

    with tile.TileContext(nc) as tc:
        from contextlib import ExitStack

        with ExitStack() as ctx:
            persist = ctx.enter_context(tc.tile_pool(name="persist", bufs=1))

            WQ = persist.tile([P, KCH, NH * DH], bf)
            WK = persist.tile([P, KCH, NH * DH], bf)
            WV = persist.tile([P, KCH, NH * DH], bf)
            WO = persist.tile([P, 2, D], bf)
            BQ = persist.tile([P, 2], f32)
            BK = persist.tile([P, 2], f32)
            BV = persist.tile([1, NH * DH], bf)
            ONES = persist.tile([1, P], bf)
            TRIM = persist.tile([P, P], bf)
            IDEN = persist.tile([P, P], bf)
            QT = persist.tile([P, 2, S], bf)
            KT = persist.tile([P, 2, S], bf)
            V = persist.tile([P, NT, NH, DH + 1], bf)
            ZN = persist.tile([P, 2, S], bf)

            xt_ctx = ctx.enter_context(tc.tile_pool(name="xt", bufs=1))
            XT = xt_ctx.tile([P, KCH, S], bf)

            # ---- input DMAs (all HW-DGE, no casting), 4 parallel rings:
            # weights for the first sweeps ride ahead of the x chunks
            nc.sync.dma_start(IDEN, iden_d)
            nc.sync.dma_start(WQ.rearrange("p a b -> p (a b)"), wq_d)
            nc.scalar.dma_start(TRIM, trim_d)
            nc.scalar.dma_start(WK.rearrange("p a b -> p (a b)"), wk_d)
            # the gpsimd ring is the slowest (~90GB/s) -> 2 chunks only;
            # the scattered vones write goes last (not needed before AV)
            qmap = {0: nc.sync, 1: nc.scalar, 2: nc.gpsimd, 3: nc.sync,
                    4: nc.scalar, 5: nc.gpsimd, 6: nc.sync, 7: nc.scalar}
            for ch in range(KCH):
                qmap[ch].dma_start(XT[:, ch, :], xt_d[ch * P:(ch + 1) * P, :])
            nc.gpsimd.dma_start(BQ, bq_d)
            nc.gpsimd.dma_start(BK, bk_d)
            nc.gpsimd.dma_start(WV.rearrange("p a b -> p (a b)"), wv_d)
            nc.gpsimd.dma_start(BV, bv_d)
            nc.gpsimd.dma_start(ONES, ones_d)
            nc.gpsimd.dma_start(WO.rearrange("p a b -> p (a b)"), wo_d)
            nc.gpsimd.dma_start(V[:, :, :, DH:DH + 1], vones_d)

            # strip-group score PSUM (2 tiles x 2 banks) and exp'd strips
            # SBUF ring (32 x 2KB/partition)
            sc_pool = ctx.enter_context(
                tc.tile_pool(name="sc", bufs=2, space="PSUM"))
            sbr_pool = ctx.enter_context(tc.tile_pool(name="sbr", bufs=44))
            nrm_pool = ctx.enter_context(tc.tile_pool(name="nrm", bufs=4))

            strips = {}       # (qc, t, kb) -> sb tile

            def emit_scores(qc, t, kbs):
                """Pair-packed scores + mask + exp for strips (qc, t, kb)."""
                for kb in kbs:
                    off = max(0, kb - 4 * qc) * P
                    diag = kb >= 4 * qc
                    g = sc_pool.tile([P, 2, 512], mybir.dt.float32, tag="g",
                                     name=f"g_{qc}_{t}_{kb}")
                    for s, pb in ((0, 0), (1, 64)):
                        nc.tensor.matmul(
                            g[:, s, off:512],
                            KT[pb:pb + 64, t, kb * P:(kb + 1) * P],
                            QT[pb:pb + 64, t, qc * 512 + off:(qc + 1) * 512],
                            start=True, stop=not diag)
                    if diag:
                        for s in range(2):
                            nc.tensor.matmul(
                                g[:, s, off:off + P], IDEN, TRIM,
                                start=False, stop=True, skip_group_check=True)
                    sb = sbr_pool.tile([P, 2, 512], bf, tag="sb",
                                       name=f"sb_{qc}_{t}_{kb}")
                    nc.scalar.activation(sb[:, :, off:512], g[:, :, off:512],
                                         Exp)
                    strips[(qc, t, kb)] = sb

            def g_av_norm(qc, t, av_pool, chunk=4):
                """AV accumulation + normalization for (qc, t); yields every
                `chunk` kb strips so callers can interleave PE work."""
                av = [av_pool.tile([DH + 1, 512], mybir.dt.float32, tag="av",
                                   name=f"av_{qc}_{t}_{s}") for s in range(2)]
                last = 4 * qc + 3
                for kb in range(4 * qc + 4):
                    off = max(0, kb - 4 * qc) * P
                    sb = strips.pop((qc, t, kb))
                    for s in range(2):
                        nc.tensor.matmul(
                            av[s][:, off:512],
                            V[:, kb, 2 * t + s, :],
                            sb[:, s, off:512],
                            start=(kb == 0), stop=(kb == last))
                    if kb % chunk == chunk - 1 and kb != last:
                        yield
                # both recip chains first, then both broadcasts, then both
                # multiplies: keeps the DVE queue from blocking on the Pool
                # broadcast mid-chain
                rrs, rdbs = [], []
                for s in range(2):
                    rd = nrm_pool.tile([1, 512], mybir.dt.float32, tag="rd",
                                       name=f"rd_{qc}_{t}_{s}")
                    nc.vector.tensor_copy(rd, av[s][DH:DH + 1, :])
                    rr = nrm_pool.tile([1, 512], mybir.dt.float32, tag="rr",
                                       name=f"rr_{qc}_{t}_{s}")
                    nc.vector.reciprocal_approx_fast(out=rr, in_=rd)
                    rrs.append(rr)
                for s in range(2):
                    rdb = nrm_pool.tile([64, 512], mybir.dt.float32, tag="rdb",
                                        name=f"rdb_{qc}_{t}_{s}")
                    nc.gpsimd.partition_broadcast(rdb, rrs[s])
                    rdbs.append(rdb)
                for s in range(2):
                    nc.vector.tensor_tensor(
                        ZN[s * 64:(s + 1) * 64, t, qc * 512:(qc + 1) * 512],
                        av[s][0:DH, :], rdbs[s], mult)
                yield

            def emit_av_norm(qc, t, av_pool):
                for _ in g_av_norm(qc, t, av_pool, chunk=99):
                    pass

            def g_outproj(qc, op_pool, ob_pool, act_evac=False):
                """Out-projection for q-chunk qc; yields per (qt, dc) tile.
                Evacuations go to DVE unless act_evac (tail: ACT is free)."""
                i = 0
                for qt in range(4 * qc, 4 * qc + 4):
                    for dc in range(2):
                        ps = op_pool.tile([P, 512], mybir.dt.float32, tag="op",
                                          name=f"op_{qt}_{dc}")
                        for t in range(2):
                            nc.tensor.matmul(
                                ps, ZN[:, t, qt * P:(qt + 1) * P],
                                WO[:, t, dc * 512:(dc + 1) * 512],
                                start=(t == 0), stop=(t == 1))
                        ob = ob_pool.tile([P, 512], bf, tag="ob",
                                          name=f"ob_{qt}_{dc}")
                        if act_evac and i % 2 == 0:
                            nc.scalar.activation(ob, ps, Copy)
                        else:
                            nc.vector.tensor_copy(ob, ps)
                        oeng = (nc.sync, nc.gpsimd)[i % 2]
                        oeng.dma_start(
                            out_d[qt * P:(qt + 1) * P,
                                  dc * 512:(dc + 1) * 512], ob)
                        i += 1
                        yield

            def emit_outproj(qc, op_pool, ob_pool):
                for _ in g_outproj(qc, op_pool, ob_pool):
                    pass

            def step(g, n=1):
                for _ in range(n):
                    if next(g, "end") == "end":
                        return

            def weave(groups, others, n_quanta):
                """Emit score groups one at a time with other-PE-work quanta
                spread evenly between them (no trailing flush blocks).
                Keeps the exp conveyor fed: ACT can only run 2 exps ahead
                (score-PSUM ring), so PE must never sit in a long foreign
                block while a score slot is open."""
                others = list(others)
                oi = [0]

                def take():
                    while oi[0] < len(others):
                        if next(others[oi[0]], "end") == "end":
                            oi[0] += 1
                        else:
                            return
                n_g = len(groups)
                done = 0
                for gi, (qc_, t_, kb_) in enumerate(groups):
                    emit_scores(qc_, t_, [kb_])
                    target = (n_quanta * (gi + 1) + n_g - 1) // n_g
                    while done < target:
                        take()
                        done += 1
                for g in others:
                    step(g, 999)

            # ======= phase 1: QK sweeps woven with qc0/qc1 scores =======
            with tc.tile_pool(name="qk_ps", bufs=4, space="PSUM") as qk_ps:
                def g_sweep(qc):
                    pst = {}
                    for wi in range(2):
                        for t in range(2):
                            pst[(wi, t)] = qk_ps.tile(
                                [P, 512], mybir.dt.float32, tag="qk",
                                name=f"qk_{qc}_{wi}_{t}")
                    if qc == 0:
                        # PE warmup (HAM) while the first DMAs stream
                        for _ in range(36):
                            nc.tensor.matmul(pst[(0, 0)][:, 0:P], IDEN, IDEN,
                                             start=True, stop=True)
                    for ch in range(KCH):
                        for wi, W_ in ((0, WQ), (1, WK)):
                            for t in range(2):
                                nc.tensor.matmul(
                                    pst[(wi, t)],
                                    W_[:, ch, t * P:(t + 1) * P],
                                    XT[:, ch, qc * 512:(qc + 1) * 512],
                                    start=(ch == 0), stop=(ch == KCH - 1))
                        yield
                    sl = slice(qc * 512, (qc + 1) * 512)
                    for t in range(2):
                        nc.vector.tensor_scalar_add(
                            QT[:, t, sl], pst[(0, t)], BQ[:, t:t + 1])
                        nc.vector.tensor_scalar_add(
                            KT[:, t, sl], pst[(1, t)], BK[:, t:t + 1])

                step(g_sweep(0), 999)
                weave([(0, t, kb) for t in range(2) for kb in range(4)],
                      [g_sweep(1)], 8)
                weave([(1, 0, kb) for kb in range(8)], [g_sweep(2)], 8)
                weave([(1, 1, kb) for kb in range(8)], [g_sweep(3)], 8)

            # == phase 2: V proj (kt quanta) + qc0/1 AV+norm + qc2 scores ==
            av_pool = ctx.enter_context(
                tc.tile_pool(name="av", bufs=2, space="PSUM"))

            with tc.tile_pool(name="v_ps", bufs=2, space="PSUM") as v_ps:
                def g_v(kts):
                    for kt in kts:
                        pv = v_ps.tile([P, NH, DH], mybir.dt.float32, tag="v",
                                       name=f"v_{kt}")
                        for ch in range(KCH):
                            nc.tensor.matmul(
                                pv, XT[:, ch, kt * P:(kt + 1) * P],
                                WV[:, ch, :], start=(ch == 0), stop=False)
                        nc.tensor.matmul(pv, ONES, BV, start=False, stop=True)
                        nc.vector.tensor_copy(V[:, kt, :, 0:DH], pv)
                        yield

                weave([(2, t, kb) for t in range(2) for kb in range(12)],
                      [g_v(range(0, 4)),
                       g_av_norm(0, 0, av_pool, chunk=2),
                       g_av_norm(0, 1, av_pool, chunk=2),
                       g_v(range(4, 8)),
                       g_av_norm(1, 0, av_pool, chunk=2),
                       g_av_norm(1, 1, av_pool, chunk=2),
                       g_v(range(8, 16))], 28)

            # ============ phase 3: qc3 scores + AV + outproj ============
            op_pool = ctx.enter_context(
                tc.tile_pool(name="op", bufs=2, space="PSUM"))
            ob_pool = ctx.enter_context(tc.tile_pool(name="ob", bufs=4))

            weave([(3, 0, kb) for kb in range(16)],
                  [g_outproj(0, op_pool, ob_pool),
                   g_av_norm(2, 0, av_pool, chunk=2),
                   g_outproj(1, op_pool, ob_pool)], 22)
            # AV(2,1) rides early in the t1 weave (its strips' ring buffers
            # are reused by sc(3,1) kb8+); AV(3,0) follows
            weave([(3, 1, kb) for kb in range(10)],
                  [g_av_norm(2, 1, av_pool, chunk=2),
                   g_av_norm(3, 0, av_pool, chunk=2)], 14)
            # tail: AV(3,1) chases its scores at a 2-kb lag while
            # outproj(2) (no exp dependency left) fills the PE gaps
            av31 = g_av_norm(3, 1, av_pool, chunk=2)
            op2 = g_outproj(2, op_pool, ob_pool)
            for kb in range(10, 16):
                emit_scores(3, 1, [kb])
                step(av31)
                step(op2)
            step(av31, 999)
            step(op2, 999)
            step(g_outproj(3, op_pool, ob_pool, act_evac=True), 999)

    nc.compile()
    return nc


def _get_nc():
    if "nc" not in _CACHE:
        _CACHE["nc"] = _build_nc()
    return _CACHE["nc"]


def _host_inputs(x, W_Q, W_K, W_V, W_O, b_Q, b_K, b_V):
    """Build the 8 per-core input maps (bf16 host-side preprocessing)."""
    x = np.asarray(x, dtype=np.float32)
    scale = 1.0 / np.sqrt(np.float32(DH))
    ones = np.ones((1, P), dtype=BF)
    vones = np.ones((P, NT * NH), dtype=BF)
    tri_mask = np.arange(P)[:, None] <= np.arange(P)[None, :]
    trim = np.where(tri_mask, np.float32(0.0),
                    np.float32(MASK_VAL)).astype(BF)
    iden = np.eye(P, dtype=BF)

    xts = [np.ascontiguousarray(x[b].T).astype(BF) for b in range(B)]

    in_maps = []
    for c in range(NCORES):
        b, hg = divmod(c, NCORES // B)
        h0 = NH * hg

        def chunked(a):   # [D, M] -> [128, KCH*M] rows p, cols (ch, m)
            return np.ascontiguousarray(
                a.reshape(KCH, P, -1).transpose(1, 0, 2).reshape(P, -1))

        wq = chunked((np.asarray(W_Q[h0:h0 + NH], np.float32) * scale)
                     .reshape(NH * DH, D).T).astype(BF)
        wk = chunked(np.asarray(W_K[h0:h0 + NH], np.float32)
                     .reshape(NH * DH, D).T).astype(BF)
        wv = chunked(np.asarray(W_V[h0:h0 + NH], np.float32)
                     .reshape(NH * DH, D).T).astype(BF)
        wo_flat = np.asarray(W_O[h0:h0 + NH], np.float32) \
            .transpose(0, 2, 1).reshape(NH * DH, D)
        wo = np.ascontiguousarray(
            wo_flat.reshape(2, P, D).transpose(1, 0, 2).reshape(P, 2 * D)) \
            .astype(BF)
        # per-partition bias columns: partition p of pair t = head
        # 2t + (p>=64), dh p%64
        bq_h = (np.asarray(b_Q[h0:h0 + NH], np.float32) * scale)
        bk_h = np.asarray(b_K[h0:h0 + NH], np.float32)
        bq = np.stack([bq_h[2 * t:2 * t + 2].reshape(P) for t in range(2)],
                      axis=1).astype(np.float32)
        bk = np.stack([bk_h[2 * t:2 * t + 2].reshape(P) for t in range(2)],
                      axis=1).astype(np.float32)
        bv = np.asarray(b_V[h0:h0 + NH], np.float32).reshape(1, NH * DH) \
            .astype(BF)
        in_maps.append({
            "xt": xts[b], "wq": wq, "wk": wk, "wv": wv, "wo": wo,
            "bq": np.ascontiguousarray(bq), "bk": np.ascontiguousarray(bk),
            "bv": np.ascontiguousarray(bv), "ones": ones, "vones": vones,
            "trim": trim, "iden": iden,
        })
    return in_maps


def run_spmd(in_maps, **kwargs):
    from concourse import bass_utils
    nc = _get_nc()
    return bass_utils.run_bass_kernel_spmd(
        nc, in_maps, core_ids=list(range(NCORES)), **kwargs)


def kernel(x, W_Q, W_K, W_V, W_O, b_Q, b_K, b_V, b_O):
    in_maps = _host_inputs(x, W_Q, W_K, W_V, W_O, b_Q, b_K, b_V)
    res = run_spmd(in_maps)
    gpb = NCORES // B
    parts = [np.asarray(res.results[c]["out"], dtype=np.float32)
             for c in range(NCORES)]
    out = np.stack(
        [sum(parts[b * gpb + g] for g in range(gpb)) for b in range(B)],
        axis=0)
    out += np.asarray(b_O, np.float32)[None, None, :]
    return out.astype(np.float32)
